# revision 44
# baseline (speedup 1.0000x reference)
"""Trainium2 Bass kernel for nn_EulerMisorientation3D (v6).

reference math (per voxel, Bunge ZXZ Euler angles scaled by [2pi, pi, 2pi]):
    g    = euler_to_matrix(x * scale)       (3x3 rotation)
    g_h  = euler_to_matrix(x_hat * scale)
    tr   = sum_i g_h[i,i] * inv(g)[i,i]
    out  = mean( arccos(0.5*(tr-1))^2 )

Per-voxel closed form (alpha=2pi*x0, beta=pi*x1, gamma=2pi*x2):
    u = cos(2pi*s), v = cos(2pi*t) with s = x0+x2, t = x0-x2
    c = cos(pi*x1)
    P4 = 4*(1+z) = (S+2)*(1+c*ch) + D*(c+ch)
        with U2 = u*uh, V2 = v*vh, S = U2+V2, D = U2-V2, z = 0.5*(tr-1)
    theta = arccos(z) = pi/2 + 2*atan(2*sigmoid(0.5*(ln Q4 - ln P4)) - 1)
        (Q4 = 8-P4; tanh(w) = 2*sigmoid(2w)-1 folded into atan's scale/bias;
         sigmoid instead of tanh so sigmoid/arctan/square share one ACT
         table set -> 3 table loads: trig -> natural_log -> sigmoid set.)

Schedule:
  DMA: ONE HWDGE ring (SP) carrying a single sequential stream (two
    interleaved rings thrash DRAM pages: 287 vs 345 GB/s measured).
    Order: t0, x1(h0), t1..t4, x1(h1).  Tiles taper [512,512,512,384,128]
    so the post-last-byte chain runs on a tiny tile.
  GPSIMD: s = x0+x2 per tile (tensor_add).
  DVE: -t = 2*x2 - s via one STT (cos even => -t == t for v); range
    wraps into [-0.5,0.5]; batch-wide fp16 product chain per half.
  ACT: sb = sin(pi*x1 - pi/2) = -cos(beta) (one [P,2,HD] instr per
    half); su4 = sin(2pi*m) = cos(2pi*(x0+-x2)); Ln x2 / Sigmoid /
    Arctan / Square(+accum) per half.

Sharding: flattened voxel axis split over 8 cores; each core reduces its
262144 voxels to [P, 2] partial sums; host sums (fp64) and divides by N.
"""

import math

import numpy as np

import concourse.bacc as bacc
import concourse.tile as tile
from concourse.tile_rust import add_dep_helper
from concourse import mybir
from concourse.bass_utils import run_bass_kernel_spmd

F32 = mybir.dt.float32
F16 = mybir.dt.float16
AF = mybir.ActivationFunctionType
OP = mybir.AluOpType

N_CORES = 8
NVOX = 128 * 128 * 128          # 2097152 voxels
PER = NVOX // N_CORES           # 262144 voxels per core
P = 128                         # SBUF partitions
COLS = PER // P                 # 2048 free-dim columns per core
SZ = [512, 512, 512, 384, 128]  # tapered tile widths
OFF = [sum(SZ[:i]) for i in range(len(SZ))]
T = len(SZ)
NH = 2
HD = COLS // NH                 # columns per half (1024)
HT = [[0, 1], [2, 3, 4]]        # tiles per half

PI = math.pi
LN_EPS = 2e-4


def build_bass():
    nc = bacc.Bacc("TRN2", target_bir_lowering=False, debug=False,
                   num_devices=N_CORES)
    xs = nc.declare_dram_parameter("xs", [3, PER], F32, isOutput=False)
    xh = nc.declare_dram_parameter("xh", [3, PER], F32, isOutput=False)
    out = nc.declare_dram_parameter("o", [P, NH], F32, isOutput=True)

    xs_v = xs[:].rearrange("c (p q) -> p c q", p=P)
    xh_v = xh[:].rearrange("c (p q) -> p c q", p=P)

    with tile.TileContext(nc) as tc:
        with (
            tc.tile_pool(name="io", bufs=1) as io,
            tc.tile_pool(name="wk", bufs=1) as wk,
            tc.tile_pool(name="half", bufs=NH) as hp,
            tc.tile_pool(name="big", bufs=1) as big,
        ):
            acc = big.tile([P, NH], F32, tag="acc")
            x1b = big.tile([P, 2, COLS], F32, tag="x1b")
            sb = big.tile([P, 2, COLS], F16, tag="sb")

            b_mpi2 = big.tile([P, 1], F32, tag="b_mpi2")
            b_eps = big.tile([P, 1], F32, tag="b_eps")
            b_eps8 = big.tile([P, 1], F32, tag="b_eps8")
            b_m1 = big.tile([P, 1], F32, tag="b_m1")
            b_ppi2 = big.tile([P, 1], F32, tag="b_ppi2")
            nc.vector.memset(b_mpi2, -PI / 2)
            nc.vector.memset(b_eps, LN_EPS)
            nc.vector.memset(b_eps8, 8.0 + LN_EPS)
            nc.vector.memset(b_m1, -1.0)
            nc.vector.memset(b_ppi2, PI / 2)

            # ---- single-ring DMA stream ----
            in02s = []
            for j in range(T):
                in02s.append(io.tile([P, 2, 2, SZ[j]], F32,
                                     tag=f"in02_{j}", name=f"in02_{j}"))

            def pair_dma(j):
                sl = slice(OFF[j], OFF[j] + SZ[j])
                nc.sync.dma_start(out=in02s[j][:, 0, :, :],
                                  in_=xs_v[:, 0:3:2, sl])
                nc.sync.dma_start(out=in02s[j][:, 1, :, :],
                                  in_=xh_v[:, 0:3:2, sl])

            def x1_dma(h):
                hs = slice(h * HD, (h + 1) * HD)
                nc.sync.dma_start(out=x1b[:, 0, hs], in_=xs_v[:, 1, hs])
                nc.sync.dma_start(out=x1b[:, 1, hs], in_=xh_v[:, 1, hs])

            pair_dma(0)
            x1_dma(0)
            pair_dma(1)
            pair_dma(2)
            pair_dma(3)
            pair_dma(4)
            x1_dma(1)

            act_chain = []
            mods = []

            su4s = []
            for h in range(NH):
                su4s.append(hp.tile([P, 4, HD], F16, tag="su4h",
                                    name=f"su4h_{h}"))

            def sb_sins(h):
                hs = slice(h * HD, (h + 1) * HD)
                act_chain.append(nc.scalar.activation(
                    sb[:, :, hs], x1b[:, :, hs], AF.Sin,
                    bias=b_mpi2[:], scale=PI))

            # ---- trig phase ----
            for j in range(T):
                if j == 1:
                    sb_sins(0)  # after su0: x1(h0) lands just after t0
                if j == 3:
                    sb_sins(1)  # x1(h1) is in flight; lands before this
                in02 = in02s[j]
                h = 0 if j in HT[0] else 1
                ko = OFF[j] - (0 if h == 0 else HD)
                ks = slice(ko, ko + SZ[j])
                m4 = wk.tile([P, 4, SZ[j]], F32, tag=f"m4_{j}",
                             name=f"m4_{j}")
                # s = x0+x2, -t = x2-x0 (both gpsimd; cos even => -t ok).
                # Wrap shifts: s-0.75, -t+0.25 -> [-0.5,0.5];
                # sin(2pi*m) = cos(2pi*(x0+-x2)).
                nc.gpsimd.tensor_add(m4[:, 0:2, :], in02[:, :, 0, :],
                                     in02[:, :, 1, :])
                nc.gpsimd.tensor_sub(m4[:, 2:4, :], in02[:, :, 1, :],
                                     in02[:, :, 0, :])
                nc.vector.add_range_wrap(
                    m4[:, 0:2, :], m4[:, 0:2, :], -0.75, 0.5, 1.0)
                mod = nc.vector.add_range_wrap(
                    m4[:, 2:4, :], m4[:, 2:4, :], 0.25, 0.5, 1.0)
                mods.append(mod)
                act_chain.append(nc.scalar.activation(
                    su4s[h][:, :, ks], m4[:], AF.Sin, bias=0.0,
                    scale=2 * PI))

            # ---- sg (sig|pi3p) for both halves first, so only the
            # uv->pq chain remains after the last sin ----
            sgs = []
            sg_first = []
            sg_last = []
            for h in range(NH):
                hs = slice(h * HD, (h + 1) * HD)
                sg = hp.tile([P, 2, HD], F16, tag="sg", name=f"sg_{h}")
                i0 = nc.vector.tensor_add(
                    sg[:, 0, :], sb[:, 0, hs], sb[:, 1, hs])
                nc.vector.tensor_mul(sg[:, 1, :], sb[:, 0, hs], sb[:, 1, hs])
                i2 = nc.vector.tensor_scalar(
                    sg[:, 1, :], sg[:, 1, :], 1.0, None, OP.add)
                sgs.append(sg)
                sg_first.append(i0)
                sg_last.append(i2)

            pqs = []
            uvs = []
            pq_ins = []
            for h in range(NH):
                su4h = su4s[h]
                sg = sgs[h]
                uv2 = hp.tile([P, 2, HD], F16, tag="uv2", name=f"uv2_{h}")
                svd = hp.tile([P, 2, HD], F16, tag="svd", name=f"svd_{h}")
                i_uv = nc.vector.tensor_mul(
                    uv2[:], su4h[:, 0::2, :], su4h[:, 1::2, :])
                uvs.append(i_uv)
                nc.vector.tensor_add(svd[:, 0, :], uv2[:, 0, :],
                                     uv2[:, 1, :])
                nc.vector.tensor_sub(svd[:, 1, :], uv2[:, 0, :],
                                     uv2[:, 1, :])
                ab = hp.tile([P, 2, HD], F16, tag="ab", name=f"ab_{h}")
                nc.vector.scalar_tensor_tensor(
                    ab[:, 0, :], svd[:, 0, :], 2.0, sg[:, 1, :],
                    OP.add, OP.mult)
                nc.vector.tensor_mul(ab[:, 1, :], svd[:, 1, :], sg[:, 0, :])
                pq = hp.tile([P, HD], F16, tag="pq", name=f"pq_{h}")
                i_pq = nc.vector.tensor_sub(pq[:], ab[:, 0, :], ab[:, 1, :])
                pqs.append(pq)
                pq_ins.append(i_pq)

            # ---- tail per half ----
            lns, dds = [], []
            for h in range(NH):
                ln = hp.tile([P, 2, HD], F16, tag="ln", name=f"ln_{h}")
                act_chain.append(nc.scalar.activation(
                    ln[:, 0, :], pqs[h][:], AF.Ln, bias=b_eps[:],
                    scale=1.0))
                act_chain.append(nc.scalar.activation(
                    ln[:, 1, :], pqs[h][:], AF.Ln, bias=b_eps8[:],
                    scale=-1.0))
                lns.append(ln)
            for h in range(NH):
                dd = hp.tile([P, HD], F16, tag="dd", name=f"dd_{h}")
                nc.vector.tensor_sub(dd[:], lns[h][:, 1, :], lns[h][:, 0, :])
                dds.append(dd)
            for h in range(NH):
                act_chain.append(nc.scalar.activation(
                    dds[h][:], dds[h][:], AF.Sigmoid, bias=0.0, scale=0.5))
            for h in range(NH):
                act_chain.append(nc.scalar.activation(
                    dds[h][:], dds[h][:], AF.Arctan, bias=b_m1[:],
                    scale=2.0))
            for h in range(NH):
                act_chain.append(nc.scalar.activation(
                    dds[h][:], dds[h][:], AF.Square, bias=b_ppi2[:],
                    scale=2.0, accum_out=acc[:, h:h + 1]))

            # ACT queue order: sins (trig) -> lns (natural_log) ->
            # sigmoid/atan/square (sigmoid_and_others).
            for a, b in zip(act_chain, act_chain[1:]):
                add_dep_helper(b.ins, a.ins, sync=False,
                               reason="ACT table-set ordering")
            # DVE order: sg-h0 after the last tile's wrap (no HOL on the
            # trig pipeline), h0 products after sg-h0, sg-h1 after pq-h0.
            add_dep_helper(sg_first[0].ins, mods[2].ins, sync=False,
                           reason="sg-h0 fills DVE idle after t2 wraps")
            add_dep_helper(uvs[0].ins, mods[T - 1].ins, sync=False,
                           reason="h0 products behind last wrap")
            add_dep_helper(sg_first[1].ins, pq_ins[0].ins, sync=False,
                           reason="sg-h1 behind pq-h0")

            nc.sync.dma_start(out=out[:], in_=acc[:])

    nc.compile()
    return nc


_CACHE = {}


def _get_nc():
    if "nc" not in _CACHE:
        _CACHE["nc"] = build_bass()
    return _CACHE["nc"]


def _run(x, x_hat, **spmd_kwargs):
    x = np.ascontiguousarray(np.asarray(x, dtype=np.float32).reshape(3, NVOX))
    xh = np.ascontiguousarray(np.asarray(x_hat, dtype=np.float32).reshape(3, NVOX))

    in_maps = []
    for c in range(N_CORES):
        sl = slice(c * PER, (c + 1) * PER)
        in_maps.append({
            "xs": np.ascontiguousarray(x[:, sl]),
            "xh": np.ascontiguousarray(xh[:, sl]),
        })

    nc = _get_nc()
    res = run_bass_kernel_spmd(
        nc, in_maps, core_ids=list(range(N_CORES)), **spmd_kwargs)
    total = 0.0
    for r in res.results:
        total += r["o"].astype(np.float64).sum()
    return np.float32(total / NVOX), res


def kernel(x: np.ndarray, x_hat: np.ndarray) -> np.ndarray:
    val, _ = _run(x, x_hat)
    return val
